# revision 1
# baseline (speedup 1.0000x reference)
"""Trainium2 Bass kernel for nn_Conv2d: x[32,128,56,56] * W[256,128,3,3] + b -> [32,256,56,56].

Stride 1, padding 1, dilation 1. Data-parallel over batch across 8 NeuronCores
(4 images per core, no collectives). Per core the conv is one accumulation
group of 9 matmuls per output tile (one per kernel tap):
PSUM[cout_chunk=128, R*56] += matmul(lhsT=Wt[tap][cin, cout_chunk],
rhs=shifted window of the zero-padded input row-block [cin=128, R+2, 58]).
Bias is fused into the PSUM->SBUF drain on the scalar engine.

Self-contained: hardcodes shapes; host-side pre-pads/retiles x and
pre-transposes W so every device DMA is contiguous.
"""

import numpy as np

B, CIN, H, W_ = 32, 128, 56, 56
COUT, KH, KW = 256, 3, 3
NCORES = 8
BPC = B // NCORES          # images per core
R = 8                      # output rows per tile -> matmul free dim R*56 = 448
NT = H // R                # row tiles per image
NPIX = R * W_              # 448
HP, WP = H + 2, W_ + 2     # padded 58x58

# "float32" = exact fp32 (4 cycles/row on PE). "float32r" = TF32-like
# single-pass mode (1 cycle/row at N>=256, ~1e-4 absmax relative error).
MM_DTYPE = "float32"

_cache = {}


def _build(mm_dtype_name):
    import concourse.mybir as mybir
    import concourse.tile as tile
    from concourse import bacc

    dt = mybir.dt
    mmdt = getattr(dt, mm_dtype_name)

    nc = bacc.Bacc("TRN2", target_bir_lowering=False, debug=False)

    # x arrives host-pre-padded per row-tile: [image, row_tile, cin, R+2, 58]
    # (zero border baked in, halo rows duplicated) so every x DMA is one
    # fully contiguous 290KB copy and the kernel needs no memsets.
    x_d = nc.dram_tensor(
        "x", [BPC, NT, CIN, R + 2, WP], mmdt, kind="ExternalInput"
    )
    # [chunk, cin, tap, cout_slice]: one contiguous 0.59MB DMA per cout chunk
    wt_d = nc.dram_tensor(
        "wt", [COUT // 128, CIN, KH * KW, 128], mmdt, kind="ExternalInput"
    )
    b_d = nc.dram_tensor("bias", [128, COUT // 128], dt.float32, kind="ExternalInput")
    o_d = nc.dram_tensor("out", [BPC, COUT, H, W_], dt.float32, kind="ExternalOutput")

    with tile.TileContext(nc) as tc:
        with (
            tc.tile_pool(name="const", bufs=1) as const_pool,
            tc.tile_pool(name="xin", bufs=1) as xin_pool,
            tc.tile_pool(name="outp", bufs=4) as out_pool,
            tc.tile_pool(name="psum", bufs=4, space="PSUM") as psum_pool,
        ):
            # One input tile per (image, row-tile): rows h0-1..h0+R of the
            # padded image (R+2 rows x 58 cols). Separate logical tiles keep
            # Tile's dependency tracking fine-grained: the first matmul group
            # only waits on its own ~290KB DMA, not all of x. Halo rows are
            # duplicated host-side (25% extra x traffic; DMA is far from the
            # bottleneck). All BPC*NT tiles stay resident (~65KB/partition).
            xt = {}

            def load_x_tile(n, ht):
                t = xin_pool.tile([CIN, R + 2, WP], mmdt, tag=f"x{n}_{ht}")
                xt[(n, ht)] = t
                nc.sync.dma_start(t[:], x_d[n, ht])

            # DMA issue order tracks the first matmul group's critical path:
            # first x tile, then chunk-0 weights, then everything else.
            load_x_tile(0, 0)
            w_t = const_pool.tile([CIN, COUT // 128, KH * KW, 128], mmdt)
            nc.sync.dma_start(w_t[:, 0], wt_d[0])
            load_x_tile(0, 1)
            nc.sync.dma_start(w_t[:, 1], wt_d[1])
            b_t = const_pool.tile([128, COUT // 128], dt.float32)
            nc.sync.dma_start(b_t[:], b_d[:])
            for n in range(BPC):
                for ht in range(NT):
                    if (n, ht) not in xt:
                        load_x_tile(n, ht)

            for n in range(BPC):
                for ht in range(NT):
                    t = xt[(n, ht)]
                    for c in range(COUT // 128):
                        p = psum_pool.tile([128, R, W_], dt.float32, tag="ps")
                        for kh in range(KH):
                            for kw in range(KW):
                                pos = kh * KW + kw
                                nc.tensor.matmul(
                                    p[:],
                                    w_t[:, c, pos],
                                    t[:, kh : kh + R, kw : kw + W_],
                                    start=(pos == 0),
                                    stop=(pos == KH * KW - 1),
                                )
                        ot = out_pool.tile([128, R, W_], dt.float32, tag="ot")
                        nc.scalar.activation(
                            ot[:],
                            p[:],
                            mybir.ActivationFunctionType.Identity,
                            bias=b_t[:, c : c + 1],
                        )
                        nc.sync.dma_start(
                            o_d[n, c * 128 : (c + 1) * 128, ht * R : ht * R + R, :],
                            ot[:],
                        )

    nc.compile()
    return nc


def _make_in_maps(x, W, b):
    x = np.asarray(x, dtype=np.float32)
    W = np.asarray(W, dtype=np.float32)
    b = np.asarray(b, dtype=np.float32)

    # Pre-pad and re-tile x: [B, CIN, 56, 56] -> [B, NT, CIN, R+2, 58] where
    # row-tile ht holds padded rows h0..h0+R+1 (zero border baked in).
    xpad = np.zeros((B, CIN, HP, WP), dtype=np.float32)
    xpad[:, :, 1 : H + 1, 1 : W_ + 1] = x
    xt = np.empty((B, NT, CIN, R + 2, WP), dtype=np.float32)
    for ht in range(NT):
        xt[:, ht] = xpad[:, :, ht * R : ht * R + R + 2, :]

    # [cout, cin, kh, kw] -> [cout_chunk, cin, kh*kw, cout_slice], contiguous
    wt = np.ascontiguousarray(
        W.reshape(COUT // 128, 128, CIN, KH * KW).transpose(0, 2, 3, 1)
    )
    bh = np.ascontiguousarray(b.reshape(COUT // 128, 128).T)

    return [
        {
            "x": xt[core * BPC : (core + 1) * BPC],
            "wt": wt,
            "bias": bh,
        }
        for core in range(NCORES)
    ]


def kernel(x, W, b):
    from concourse.bass_utils import run_bass_kernel_spmd

    if MM_DTYPE not in _cache:
        _cache[MM_DTYPE] = _build(MM_DTYPE)
    nc = _cache[MM_DTYPE]

    in_maps = _make_in_maps(x, W, b)
    try:
        res = run_bass_kernel_spmd(nc, in_maps, list(range(NCORES))).results
    except Exception:
        # A prior session can leave the accelerator in a transient
        # unrecoverable state; one retry after re-init clears it.
        import time

        time.sleep(15)
        res = run_bass_kernel_spmd(nc, in_maps, list(range(NCORES))).results
    return np.concatenate([res[i]["out"] for i in range(NCORES)], axis=0)



# revision 2
# speedup vs baseline: 2.8940x; 2.8940x over previous
"""Trainium2 Bass kernel for nn_Conv2d: x[32,128,56,56] * W[256,128,3,3] + b -> [32,256,56,56].

Stride 1, padding 1, dilation 1. Data-parallel over batch across 8 NeuronCores
(4 images per core, no collectives). Per core the conv is one accumulation
group of 9 matmuls per output tile (one per kernel tap):
PSUM[cout_chunk=128, R*56] += matmul(lhsT=Wt[tap][cin, cout_chunk],
rhs=shifted window of the zero-padded input row-block [cin=128, R+2, 58]).
Bias is fused into the PSUM->SBUF drain on the scalar engine.

Self-contained: hardcodes shapes; host-side pre-pads/retiles x and
pre-transposes W so every device DMA is contiguous.
"""

import numpy as np

B, CIN, H, W_ = 32, 128, 56, 56
COUT, KH, KW = 256, 3, 3
NCORES = 8
BPC = B // NCORES          # images per core
R = 8                      # output rows per tile -> matmul free dim R*56 = 448
NT = H // R                # row tiles per image
NPIX = R * W_              # 448
HP, WP = H + 2, W_ + 2     # padded 58x58

# "float32" = exact fp32 (4 cycles/row on PE). "float32r" = TF32-like
# single-pass mode (1 cycle/row at N>=256, ~1e-4 absmax relative error).
MM_DTYPE = "float32r"

_cache = {}


def _build(mm_dtype_name):
    import concourse.mybir as mybir
    import concourse.tile as tile
    from concourse import bacc

    dt = mybir.dt
    mmdt = getattr(dt, mm_dtype_name)

    nc = bacc.Bacc("TRN2", target_bir_lowering=False, debug=False)

    # x arrives host-pre-padded per row-tile: [image, row_tile, cin, R+2, 58]
    # (zero border baked in, halo rows duplicated) so every x DMA is one
    # fully contiguous 290KB copy and the kernel needs no memsets.
    x_d = nc.dram_tensor(
        "x", [BPC, NT, CIN, R + 2, WP], mmdt, kind="ExternalInput"
    )
    # [chunk, cin, tap, cout_slice]: one contiguous 0.59MB DMA per cout chunk
    wt_d = nc.dram_tensor(
        "wt", [COUT // 128, CIN, KH * KW, 128], mmdt, kind="ExternalInput"
    )
    b_d = nc.dram_tensor("bias", [128, COUT // 128], dt.float32, kind="ExternalInput")
    o_d = nc.dram_tensor("out", [BPC, COUT, H, W_], dt.float32, kind="ExternalOutput")

    with tile.TileContext(nc) as tc:
        with (
            tc.tile_pool(name="const", bufs=1) as const_pool,
            tc.tile_pool(name="xin", bufs=1) as xin_pool,
            tc.tile_pool(name="outp", bufs=4) as out_pool,
            tc.tile_pool(name="psum", bufs=4, space="PSUM") as psum_pool,
        ):
            # One input tile per (image, row-tile): rows h0-1..h0+R of the
            # padded image (R+2 rows x 58 cols). Separate logical tiles keep
            # Tile's dependency tracking fine-grained: the first matmul group
            # only waits on its own ~290KB DMA, not all of x. Halo rows are
            # duplicated host-side (25% extra x traffic; DMA is far from the
            # bottleneck). All BPC*NT tiles stay resident (~65KB/partition).
            xt = {}

            def load_x_tile(n, ht):
                t = xin_pool.tile([CIN, R + 2, WP], mmdt, tag=f"x{n}_{ht}")
                xt[(n, ht)] = t
                nc.sync.dma_start(t[:], x_d[n, ht])

            # DMA issue order tracks the first matmul group's critical path:
            # first x tile, then chunk-0 weights, then everything else.
            load_x_tile(0, 0)
            w_t = const_pool.tile([CIN, COUT // 128, KH * KW, 128], mmdt)
            nc.sync.dma_start(w_t[:, 0], wt_d[0])
            load_x_tile(0, 1)
            nc.sync.dma_start(w_t[:, 1], wt_d[1])
            b_t = const_pool.tile([128, COUT // 128], dt.float32)
            nc.sync.dma_start(b_t[:], b_d[:])
            for n in range(BPC):
                for ht in range(NT):
                    if (n, ht) not in xt:
                        load_x_tile(n, ht)

            for n in range(BPC):
                for ht in range(NT):
                    t = xt[(n, ht)]
                    for c in range(COUT // 128):
                        p = psum_pool.tile([128, R, W_], dt.float32, tag="ps")
                        for kh in range(KH):
                            for kw in range(KW):
                                pos = kh * KW + kw
                                nc.tensor.matmul(
                                    p[:],
                                    w_t[:, c, pos],
                                    t[:, kh : kh + R, kw : kw + W_],
                                    start=(pos == 0),
                                    stop=(pos == KH * KW - 1),
                                )
                        ot = out_pool.tile([128, R, W_], dt.float32, tag="ot")
                        nc.scalar.activation(
                            ot[:],
                            p[:],
                            mybir.ActivationFunctionType.Identity,
                            bias=b_t[:, c : c + 1],
                        )
                        nc.sync.dma_start(
                            o_d[n, c * 128 : (c + 1) * 128, ht * R : ht * R + R, :],
                            ot[:],
                        )

    nc.compile()
    return nc


def _make_in_maps(x, W, b):
    x = np.asarray(x, dtype=np.float32)
    W = np.asarray(W, dtype=np.float32)
    b = np.asarray(b, dtype=np.float32)

    # Pre-pad and re-tile x: [B, CIN, 56, 56] -> [B, NT, CIN, R+2, 58] where
    # row-tile ht holds padded rows h0..h0+R+1 (zero border baked in).
    xpad = np.zeros((B, CIN, HP, WP), dtype=np.float32)
    xpad[:, :, 1 : H + 1, 1 : W_ + 1] = x
    xt = np.empty((B, NT, CIN, R + 2, WP), dtype=np.float32)
    for ht in range(NT):
        xt[:, ht] = xpad[:, :, ht * R : ht * R + R + 2, :]

    # [cout, cin, kh, kw] -> [cout_chunk, cin, kh*kw, cout_slice], contiguous
    wt = np.ascontiguousarray(
        W.reshape(COUT // 128, 128, CIN, KH * KW).transpose(0, 2, 3, 1)
    )
    bh = np.ascontiguousarray(b.reshape(COUT // 128, 128).T)

    return [
        {
            "x": xt[core * BPC : (core + 1) * BPC],
            "wt": wt,
            "bias": bh,
        }
        for core in range(NCORES)
    ]


def kernel(x, W, b):
    from concourse.bass_utils import run_bass_kernel_spmd

    if MM_DTYPE not in _cache:
        _cache[MM_DTYPE] = _build(MM_DTYPE)
    nc = _cache[MM_DTYPE]

    in_maps = _make_in_maps(x, W, b)
    try:
        res = run_bass_kernel_spmd(nc, in_maps, list(range(NCORES))).results
    except Exception:
        # A prior session can leave the accelerator in a transient
        # unrecoverable state; one retry after re-init clears it.
        import time

        time.sleep(15)
        res = run_bass_kernel_spmd(nc, in_maps, list(range(NCORES))).results
    return np.concatenate([res[i]["out"] for i in range(NCORES)], axis=0)



# revision 3
# speedup vs baseline: 3.4429x; 1.1896x over previous
"""Trainium2 Bass kernel for nn_Conv2d: x[32,128,56,56] * W[256,128,3,3] + b -> [32,256,56,56].

Stride 1, padding 1, dilation 1. Data-parallel over batch across 8 NeuronCores
(4 images per core, no collectives). Per core the conv is one accumulation
group of 9 matmuls per output tile (one per kernel tap):
PSUM[cout_chunk=128, R*56] += matmul(lhsT=Wt[tap][cin, cout_chunk],
rhs=shifted window of the zero-padded input row-block [cin=128, R+2, 58]).
Bias is fused into the PSUM->SBUF drain on the scalar engine.

Matmuls run in bf16 (1 PE cycle/row vs 4 for exact fp32; enables fast weight
load). PSUM accumulation and the output stay fp32; measured absmax rel err is
~1e-3 vs the fp32 reference.

DMA flow is just-in-time: x row-tile DMAs are interleaved with the output
DMAs inside the main loop (prefetch depth 5) instead of bulk-issued up
front. The Sync queue triggers DMAs in order through an 8-slot completion
window, so bulk-issuing all 28 input tiles parks every output DMA behind
~8 MB of input traffic -> output SBUF buffers never recycle -> PSUM fills
-> the PE stalls mid-run and the HAM clock-gate re-throttles it (measured
9 us stall + 10 us at half clock). Interleaved issue keeps the PE streaming
continuously.

Self-contained: hardcodes shapes; host-side pre-pads/retiles x and
pre-transposes W so every device DMA is contiguous.
"""

import numpy as np

B, CIN, H, W_ = 32, 128, 56, 56
COUT, KH, KW = 256, 3, 3
NCORES = 8
BPC = B // NCORES          # images per core
R = 8                      # output rows per tile -> matmul free dim R*56 = 448
NT = H // R                # row tiles per image
NTILE = BPC * NT
HP, WP = H + 2, W_ + 2     # padded 58x58
NCH = COUT // 128          # cout chunks (2)

MM_DTYPE = "bfloat16"
XBUFS = 6                  # x-tile ring depth
PREFETCH = 5               # x tiles loaded ahead of consumption

_cache = {}


def _np_mm_dtype():
    if MM_DTYPE == "bfloat16":
        import ml_dtypes

        return ml_dtypes.bfloat16
    return np.float32


def _build(mm_dtype_name):
    import concourse.mybir as mybir
    import concourse.tile as tile
    from concourse import bacc

    dt = mybir.dt
    mmdt = getattr(dt, mm_dtype_name)

    nc = bacc.Bacc("TRN2", target_bir_lowering=False, debug=False)

    # x arrives host-pre-padded per row-tile: [image, row_tile, cin, R+2, 58]
    # (zero border baked in, halo rows duplicated) so every x DMA is one
    # fully contiguous copy and the kernel needs no memsets.
    x_d = nc.dram_tensor(
        "x", [BPC, NT, CIN, R + 2, WP], mmdt, kind="ExternalInput"
    )
    # [chunk, cin, tap, cout_slice]: one contiguous DMA per cout chunk
    wt_d = nc.dram_tensor("wt", [NCH, CIN, KH * KW, 128], mmdt, kind="ExternalInput")
    b_d = nc.dram_tensor("bias", [128, NCH], dt.float32, kind="ExternalInput")
    # Output laid out [image, cout%128 (partition), cout//128, h, w] so both
    # cout chunks of one row-tile go out in a single DMA; host untangles.
    o_d = nc.dram_tensor("out", [BPC, 128, NCH, H, W_], dt.float32, kind="ExternalOutput")

    with tile.TileContext(nc) as tc:
        with (
            tc.tile_pool(name="const", bufs=1) as const_pool,
            tc.tile_pool(name="xin", bufs=XBUFS) as xin_pool,
            tc.tile_pool(name="outp", bufs=4) as out_pool,
            tc.tile_pool(name="psum", bufs=8, space="PSUM") as psum_pool,
        ):
            xt = []

            def load_x(idx):
                n, ht = divmod(idx, NT)
                t = xin_pool.tile([CIN, R + 2, WP], mmdt, tag="xt")
                nc.sync.dma_start(t[:], x_d[n, ht])
                xt.append(t)

            # Critical path first: chunk-0 weights + first x tile, then the
            # rest of the constants and the prefetch window.
            w_t = const_pool.tile([CIN, NCH, KH * KW, 128], mmdt)
            nc.sync.dma_start(w_t[:, 0], wt_d[0])
            load_x(0)
            nc.sync.dma_start(w_t[:, 1], wt_d[1])
            b_t = const_pool.tile([128, NCH], dt.float32)
            nc.sync.dma_start(b_t[:], b_d[:])
            for i in range(1, PREFETCH):
                load_x(i)

            for idx in range(NTILE):
                n, ht = divmod(idx, NT)
                if idx + PREFETCH < NTILE:
                    load_x(idx + PREFETCH)
                t = xt[idx]
                ot = out_pool.tile([128, NCH, R, W_], dt.float32, tag="ot")
                for c in range(NCH):
                    p = psum_pool.tile([128, R, W_], dt.float32, tag="ps")
                    for kh in range(KH):
                        for kw in range(KW):
                            pos = kh * KW + kw
                            nc.tensor.matmul(
                                p[:],
                                w_t[:, c, pos],
                                t[:, kh : kh + R, kw : kw + W_],
                                start=(pos == 0),
                                stop=(pos == KH * KW - 1),
                            )
                    nc.scalar.activation(
                        ot[:, c],
                        p[:],
                        mybir.ActivationFunctionType.Identity,
                        bias=b_t[:, c : c + 1],
                    )
                nc.sync.dma_start(
                    o_d[n, :, :, ht * R : ht * R + R, :],
                    ot[:],
                )

    nc.compile()
    return nc


def _make_in_maps(x, W, b):
    mdt = _np_mm_dtype()
    x = np.asarray(x, dtype=np.float32)
    W = np.asarray(W, dtype=np.float32)
    b = np.asarray(b, dtype=np.float32)

    # Pre-pad and re-tile x: [B, CIN, 56, 56] -> [B, NT, CIN, R+2, 58] where
    # row-tile ht holds padded rows h0..h0+R+1 (zero border baked in).
    xpad = np.zeros((B, CIN, HP, WP), dtype=mdt)
    xpad[:, :, 1 : H + 1, 1 : W_ + 1] = x.astype(mdt)
    xt = np.empty((B, NT, CIN, R + 2, WP), dtype=mdt)
    for ht in range(NT):
        xt[:, ht] = xpad[:, :, ht * R : ht * R + R + 2, :]

    # [cout, cin, kh, kw] -> [cout_chunk, cin, kh*kw, cout_slice], contiguous
    wt = np.ascontiguousarray(
        W.reshape(NCH, 128, CIN, KH * KW).transpose(0, 2, 3, 1)
    ).astype(mdt)
    bh = np.ascontiguousarray(b.reshape(NCH, 128).T)

    return [
        {
            "x": xt[core * BPC : (core + 1) * BPC],
            "wt": wt,
            "bias": bh,
        }
        for core in range(NCORES)
    ]


def kernel(x, W, b):
    from concourse.bass_utils import run_bass_kernel_spmd

    if MM_DTYPE not in _cache:
        _cache[MM_DTYPE] = _build(MM_DTYPE)
    nc = _cache[MM_DTYPE]

    in_maps = _make_in_maps(x, W, b)
    try:
        res = run_bass_kernel_spmd(nc, in_maps, list(range(NCORES))).results
    except Exception:
        # A prior session can leave the accelerator in a transient
        # unrecoverable state; one retry after re-init clears it.
        import time

        time.sleep(15)
        res = run_bass_kernel_spmd(nc, in_maps, list(range(NCORES))).results
    # [BPC, 128, NCH, H, W] -> [BPC, NCH*128, H, W]
    outs = [
        res[i]["out"].transpose(0, 2, 1, 3, 4).reshape(BPC, COUT, H, W_)
        for i in range(NCORES)
    ]
    return np.concatenate(outs, axis=0)


# revision 6
# speedup vs baseline: 3.5008x; 1.0168x over previous
"""Trainium2 Bass kernel for nn_Conv2d: x[32,128,56,56] * W[256,128,3,3] + b -> [32,256,56,56].

Stride 1, padding 1, dilation 1. Data-parallel over batch across 8 NeuronCores
(4 images per core, no collectives). Per core the conv is one accumulation
group of 9 matmuls per output tile (one per kernel tap):
PSUM[cout_chunk=128, R*56] += matmul(lhsT=Wt[tap][cin, cout_chunk],
rhs=shifted window of the zero-padded input row-block [cin=128, R+2, 58]).
Bias is fused into the PSUM->SBUF drain on the scalar engine.

Matmuls run in bf16 (1 PE cycle/row vs 4 for exact fp32; enables fast weight
load). PSUM accumulation and the output stay fp32; measured absmax rel err is
~1e-3 vs the fp32 reference.

DMA flow is just-in-time: x row-tile DMAs are interleaved with the output
DMAs inside the main loop (prefetch depth 5) instead of bulk-issued up
front. The Sync queue triggers DMAs in order through an 8-slot completion
window, so bulk-issuing all 28 input tiles parks every output DMA behind
~8 MB of input traffic -> output SBUF buffers never recycle -> PSUM fills
-> the PE stalls mid-run and the HAM clock-gate re-throttles it (measured
9 us stall + 10 us at half clock). Interleaved issue keeps the PE streaming
continuously.

Self-contained: hardcodes shapes; host-side pre-pads/retiles x and
pre-transposes W so every device DMA is contiguous.
"""

import numpy as np

B, CIN, H, W_ = 32, 128, 56, 56
COUT, KH, KW = 256, 3, 3
NCORES = 8
BPC = B // NCORES          # images per core
R = 8                      # output rows per tile -> matmul free dim R*56 = 448
NT = H // R                # row tiles per image
NTILE = BPC * NT
HP, WP = H + 2, W_ + 2     # padded 58x58
NCH = COUT // 128          # cout chunks (2)

MM_DTYPE = "bfloat16"
XBUFS = 6                  # x-tile ring depth
PREFETCH = 5               # x tiles loaded ahead of consumption

_cache = {}


def _np_mm_dtype():
    if MM_DTYPE == "bfloat16":
        import ml_dtypes

        return ml_dtypes.bfloat16
    return np.float32


def _build(mm_dtype_name):
    import concourse.mybir as mybir
    import concourse.tile as tile
    from concourse import bacc

    dt = mybir.dt
    mmdt = getattr(dt, mm_dtype_name)

    nc = bacc.Bacc("TRN2", target_bir_lowering=False, debug=False)

    # x arrives host-pre-padded per row-tile: [image, row_tile, cin, R+2, 58]
    # (zero border baked in, halo rows duplicated) so every x DMA is one
    # fully contiguous copy and the kernel needs no memsets.
    x_d = nc.dram_tensor(
        "x", [BPC, NT, CIN, R + 2, WP], mmdt, kind="ExternalInput"
    )
    # [chunk, cin, tap, cout_slice]: one contiguous DMA per cout chunk
    wt_d = nc.dram_tensor("wt", [NCH, CIN, KH * KW, 128], mmdt, kind="ExternalInput")
    b_d = nc.dram_tensor("bias", [128, NCH], dt.float32, kind="ExternalInput")
    # Output laid out [image, cout%128 (partition), cout//128, h, w] so both
    # cout chunks of one row-tile go out in a single DMA; host untangles.
    o_d = nc.dram_tensor("out", [BPC, 128, NCH, H, W_], dt.float32, kind="ExternalOutput")

    with tile.TileContext(nc) as tc:
        with (
            tc.tile_pool(name="const", bufs=1) as const_pool,
            tc.tile_pool(name="xin", bufs=XBUFS) as xin_pool,
            tc.tile_pool(name="outp", bufs=4) as out_pool,
            tc.tile_pool(name="psum", bufs=8, space="PSUM") as psum_pool,
        ):
            xt = []

            def load_x(idx):
                n, ht = divmod(idx, NT)
                t = xin_pool.tile([CIN, R + 2, WP], mmdt, tag="xt")
                nc.sync.dma_start(t[:], x_d[n, ht])
                xt.append(t)

            # PE clock warm-up: the HAM activity monitor keeps the PE at half
            # clock until it has been busy ~3.4us. One 9-matmul group on
            # zeroed SBUF (result never read) during the initial DMA wait
            # brings it to full rate before the first real matmul.
            zw_t = const_pool.tile([CIN, 128], mmdt)
            nc.gpsimd.memset(zw_t[:], 0.0)
            zx_t = const_pool.tile([CIN, R, W_], mmdt)
            nc.gpsimd.memset(zx_t[:], 0.0)
            pw = psum_pool.tile([128, R, W_], dt.float32, tag="ps")
            for i in range(9):
                nc.tensor.matmul(
                    pw[:],
                    zw_t[:],
                    zx_t[:],
                    start=(i == 0),
                    stop=(i == 8),
                )

            # Critical path first: tap-0 of chunk-0 weights (all the first
            # matmul needs) + first x tile, then the rest of the constants
            # and the prefetch window.
            w_t = const_pool.tile([CIN, NCH, KH * KW, 128], mmdt)
            nc.sync.dma_start(w_t[:, 0, 0], wt_d[0, :, 0])
            load_x(0)
            nc.sync.dma_start(w_t[:, 0, 1:], wt_d[0, :, 1:])
            nc.sync.dma_start(w_t[:, 1], wt_d[1])
            b_t = const_pool.tile([128, NCH], dt.float32)
            nc.sync.dma_start(b_t[:], b_d[:])
            for i in range(1, PREFETCH):
                load_x(i)

            for idx in range(NTILE):
                n, ht = divmod(idx, NT)
                if idx + PREFETCH < NTILE:
                    load_x(idx + PREFETCH)
                t = xt[idx]
                ot = out_pool.tile([128, NCH, R, W_], dt.float32, tag="ot")
                for c in range(NCH):
                    p = psum_pool.tile([128, R, W_], dt.float32, tag="ps")
                    for kh in range(KH):
                        for kw in range(KW):
                            pos = kh * KW + kw
                            nc.tensor.matmul(
                                p[:],
                                w_t[:, c, pos],
                                t[:, kh : kh + R, kw : kw + W_],
                                start=(pos == 0),
                                stop=(pos == KH * KW - 1),
                            )
                    nc.scalar.activation(
                        ot[:, c],
                        p[:],
                        mybir.ActivationFunctionType.Identity,
                        bias=b_t[:, c : c + 1],
                    )
                    if idx == NTILE - 1:
                        # Tail latency: ship each chunk of the final tile as
                        # soon as its drain finishes instead of waiting for
                        # both.
                        nc.sync.dma_start(
                            o_d[n, :, c, ht * R : ht * R + R, :],
                            ot[:, c],
                        )
                if idx < NTILE - 1:
                    nc.sync.dma_start(
                        o_d[n, :, :, ht * R : ht * R + R, :],
                        ot[:],
                    )

    nc.compile()
    return nc


def _make_in_maps(x, W, b):
    mdt = _np_mm_dtype()
    x = np.asarray(x, dtype=np.float32)
    W = np.asarray(W, dtype=np.float32)
    b = np.asarray(b, dtype=np.float32)

    # Pre-pad and re-tile x: [B, CIN, 56, 56] -> [B, NT, CIN, R+2, 58] where
    # row-tile ht holds padded rows h0..h0+R+1 (zero border baked in).
    xpad = np.zeros((B, CIN, HP, WP), dtype=mdt)
    xpad[:, :, 1 : H + 1, 1 : W_ + 1] = x.astype(mdt)
    xt = np.empty((B, NT, CIN, R + 2, WP), dtype=mdt)
    for ht in range(NT):
        xt[:, ht] = xpad[:, :, ht * R : ht * R + R + 2, :]

    # [cout, cin, kh, kw] -> [cout_chunk, cin, kh*kw, cout_slice], contiguous
    wt = np.ascontiguousarray(
        W.reshape(NCH, 128, CIN, KH * KW).transpose(0, 2, 3, 1)
    ).astype(mdt)
    bh = np.ascontiguousarray(b.reshape(NCH, 128).T)

    return [
        {
            "x": xt[core * BPC : (core + 1) * BPC],
            "wt": wt,
            "bias": bh,
        }
        for core in range(NCORES)
    ]


def kernel(x, W, b):
    from concourse.bass_utils import run_bass_kernel_spmd

    if MM_DTYPE not in _cache:
        _cache[MM_DTYPE] = _build(MM_DTYPE)
    nc = _cache[MM_DTYPE]

    in_maps = _make_in_maps(x, W, b)
    try:
        res = run_bass_kernel_spmd(nc, in_maps, list(range(NCORES))).results
    except Exception:
        # A prior session can leave the accelerator in a transient
        # unrecoverable state; one retry after re-init clears it.
        import time

        time.sleep(15)
        res = run_bass_kernel_spmd(nc, in_maps, list(range(NCORES))).results
    # [BPC, 128, NCH, H, W] -> [BPC, NCH*128, H, W]
    outs = [
        res[i]["out"].transpose(0, 2, 1, 3, 4).reshape(BPC, COUT, H, W_)
        for i in range(NCORES)
    ]
    return np.concatenate(outs, axis=0)
